# revision 8
# baseline (speedup 1.0000x reference)
"""Contrastive-learning NCE loss on 8 trn2 NeuronCores (Bass/Tile).

Problem (hardcoded shapes): B=8, L=1024, D_in=512, D_feat=256, N=B*L=8192.
  emb_k = relu(feature_k @ W + b)                     [B, L, Df]
  positive = <e1,e2> + banded_diag_mean terms         [N]
  negative = logsumexp(e1 @ e2.T, axis=-1) - log(N)   [N]
  loss = mean(-positive + negative)

Sharding: token dim N split across 8 cores = one batch row each (L == N/8).
Each core computes its [1024, 8192] slab of the similarity matrix against the
full emb_2 (recomputed locally from full feature2). The host rotates feature2
per core so the core's own batch always sits at columns 0:1023 -> the SPMD
program is core-index free.

fp8 pipeline: all matmuls are fp8e4m3 MatmulPerfMode.DoubleRow (K=256/inst,
2 elem/cycle moving stream at full p-state). Host pre-scales W,b by 16 so W
leaves the fp8 subnormal range; embeddings are stored at 16x scale. Every dot
product is 256x true scale: EXP applies scale=1/256, host divides pos_* by
256. Host inputs are pre-swizzled to [128, ko, n] so every DMA line is
contiguous per partition.

The kernel is ACT-bound (~65us/core of exp). Schedule highlights:
- warmup matmuls during the DMA head lift the PE out of the HAM throttle
  (cold 1.2 GHz -> 2.4 GHz) before the first projection lands
- constant exp shift K=48 (max sim ~120 -> args <= ~72, fp32/bf16-safe), so
  no per-row diag bias and no diag DMA round-trip on the critical path
- f2 chunks 1..3 project in 1024-col halves interleaved between sim tiles;
  with a warm PE each insert (~1.7us) fits inside ACT's 2-tile buffer
- banded terms read bf16 casts of the fp8 embeddings (lossless, so the main
  positive term matches the sim diagonal exactly); casts run on the
  otherwise-idle GPSIMD, boxsums on DVE after the last projection epilogue,
  and the row-sum matmuls slot into late sim column groups
- ACT's exp table is preloaded with a dummy activation during the DMA head
"""

import numpy as np
import ml_dtypes
from contextlib import ExitStack

import concourse.bass as bass
import concourse.tile as tile
from concourse import bacc, mybir
from concourse import bass_utils

dt = mybir.dt
AF = mybir.ActivationFunctionType
ALU = mybir.AluOpType
DR = mybir.MatmulPerfMode.DoubleRow

N_CORES = 8
B, L, DIN, DF = 8, 1024, 512, 256
N = B * L
KO = DIN // 128     # 4 k-tiles of the projection contraction
PAD = 4             # box-filter padding (max supported positive_range)
LP = L + 2 * PAD    # padded row length for banded box sums
CW = 2048           # column group width of sim/EXP tiles
NCG = N // CW       # 4 column groups
SC = 16.0           # host pre-scale on W, b; emb stored at 16x
KSHIFT = 48.0       # constant exp shift (max sim ~120 -> args <= ~72)

_module_cache = {}


def _box_terms(w: int):
    """Decompose window width w (odd, <= 2*PAD+1) into power-of-2 segments:
    returns [(pow, offset), ...] s.t. window = concat of segments."""
    terms, off = [], 0
    for p in (8, 4, 2, 1):
        if w >= p:
            terms.append((p, off))
            off += p
            w -= p
    assert w == 0
    return terms


def _build(r_self: int, r_tgt: int):
    nc = bacc.Bacc("TRN2", target_bir_lowering=False, debug=False, num_devices=N_CORES)

    f1d = nc.dram_tensor("f1d", [128, KO, L], dt.float8e4, kind="ExternalInput").ap()
    f2d = nc.dram_tensor("f2d", [128, KO, N], dt.float8e4, kind="ExternalInput").ap()
    w_in = nc.dram_tensor("w_in", [128, KO, DF], dt.float8e4, kind="ExternalInput").ap()
    b_in = nc.dram_tensor("b_in", [128, 2], dt.float32, kind="ExternalInput").ap()

    pos_main = nc.dram_tensor("pos_main", [L], dt.float32, kind="ExternalOutput").ap()
    pos_self = nc.dram_tensor("pos_self", [L], dt.float32, kind="ExternalOutput").ap()
    pos_tgt = nc.dram_tensor("pos_tgt", [L], dt.float32, kind="ExternalOutput").ap()
    s_out = nc.dram_tensor("s_out", [128, 8 * NCG], dt.float32, kind="ExternalOutput").ap()

    with tile.TileContext(nc) as tc, ExitStack() as ctx:
        const = ctx.enter_context(tc.tile_pool(name="const", bufs=1))
        stage = ctx.enter_context(tc.tile_pool(name="stage", bufs=3))
        emb = ctx.enter_context(tc.tile_pool(name="emb", bufs=1))
        band = ctx.enter_context(tc.tile_pool(name="band", bufs=1))
        prodp = ctx.enter_context(tc.tile_pool(name="prodp", bufs=2))
        rows = ctx.enter_context(tc.tile_pool(name="rows", bufs=1))
        esc = ctx.enter_context(tc.tile_pool(name="esc2", bufs=2))
        mmp = ctx.enter_context(tc.tile_pool(name="mmp", bufs=2, space="PSUM"))

        # ---- staged input DMAs (f1 first: it gates the first matmul) ----
        fst_f1 = stage.tile([128, KO * L], dt.float8e4, tag="fst1")
        fst13 = fst_f1[:].rearrange("p (ko n) -> p ko n", ko=KO)
        nc.sync.dma_start(out=fst13, in_=f1d[:])
        wt = const.tile([128, KO * DF], dt.float8e4)
        wt3 = wt[:].rearrange("p (ko d) -> p ko d", ko=KO)
        nc.sync.dma_start(out=wt3, in_=w_in[:])
        b_col = const.tile([128, 2], dt.float32)
        nc.sync.dma_start(out=b_col[:], in_=b_in[:])

        # ---- constants, exp-table preload, PE warmup --------------------
        ones_f = const.tile([128, 1], dt.float32)
        nc.vector.memset(ones_f[:], 1.0)
        ones = const.tile([128, 1], dt.bfloat16)
        nc.vector.tensor_copy(ones[:], ones_f[:])
        scr = const.tile([128, 512], dt.bfloat16)
        nc.vector.memset(scr[:], 0.0)
        dum = const.tile([128, 1], dt.bfloat16)
        nkb = const.tile([128, 1], dt.float32)             # -KSHIFT bias column
        nc.vector.memset(nkb[:], -KSHIFT)
        nc.scalar.activation(dum[:], ones_f[:], AF.Exp, bias=0.0, scale=1.0)
        # ~4us of dummy matmuls lift the HAM throttle before real work lands
        for w in range(2):
            warm = mmp.tile([1, 512], dt.float32, tag="mm", name=f"warm{w}")
            for _ in range(5):
                nc.tensor.matmul(warm[:], ones[:, 0:1], scr[:], start=True, stop=True)

        # ---- embedding storage -----------------------------------------
        # fp8 at 16x scale, [128, ksub, cols] layout (ksub = 2 halves of Df)
        e1f = emb.tile([128, 2 * L], dt.float8e4, name="e1f")
        e2f = emb.tile([128, 2 * N], dt.float8e4, name="e2f")
        e1f3 = e1f[:].rearrange("p (k n) -> p k n", k=2)
        e2f3 = e2f[:].rearrange("p (k n) -> p k n", k=2)
        # padded bf16 copies for the banded terms (zeros in the pads)
        e1bp = band.tile([128, 2 * LP], dt.bfloat16, name="e1bp")
        e2bp = band.tile([128, 2 * LP], dt.bfloat16, name="e2bp")
        for t in (e1bp, e2bp):
            for d in range(2):
                nc.gpsimd.memset(t[:, d * LP: d * LP + PAD], 0.0)
                nc.gpsimd.memset(t[:, d * LP + PAD + L: (d + 1) * LP], 0.0)

        def project(src3, col0, ncols, dstf, dstride, staged=None):
            """DoubleRow-project pre-swizzled cols [col0, col0+ncols) into the
            fp8 tile dstf at flat offset d*dstride+col0 per k-subtile d."""
            if staged is None:
                fst = stage.tile([128, KO * ncols], dt.float8e4, tag=f"fst{ncols}")
                fst3 = fst[:].rearrange("p (ko n) -> p ko n", ko=KO)
                nc.sync.dma_start(out=fst3, in_=src3[:, :, col0:col0 + ncols])
            else:
                fst3 = staged
            for d in range(2):
                ps = mmp.tile([128, ncols], dt.float32, tag="mm", name=f"pj{col0}_{d}")
                for kop in range(KO // 2):
                    for h in range(ncols // 512):
                        nc.tensor.matmul(
                            ps[:, h * 512:(h + 1) * 512],
                            wt3[:, 2 * kop:2 * kop + 2, d * 128:(d + 1) * 128],
                            fst3[:, 2 * kop:2 * kop + 2, h * 512:(h + 1) * 512],
                            start=(kop == 0), stop=(kop == KO // 2 - 1),
                            perf_mode=DR)
                nc.vector.tensor_scalar(
                    dstf[:, d * dstride + col0: d * dstride + col0 + ncols], ps[:],
                    b_col[:, d:d + 1], 0.0, ALU.add, ALU.max)

        # ---- sim tile: 4 DR matmuls + one EXP with row-sum accumulate ---
        stot = const.tile([128, 8 * NCG], dt.float32)

        def sim_tile(m, c):
            ps = mmp.tile([128, CW], dt.float32, tag="mm", name=f"sim{m}_{c}")
            for q in range(CW // 512):
                nc.tensor.matmul(
                    ps[:, q * 512:(q + 1) * 512],
                    e1f3[:, :, m * 128:(m + 1) * 128],
                    e2f3[:, :, c * CW + q * 512: c * CW + (q + 1) * 512],
                    start=True, stop=True, perf_mode=DR)
            ex = esc.tile([128, CW], dt.bfloat16, tag="ex")
            nc.scalar.activation(ex[:], ps[:], AF.Exp,
                                 bias=nkb[:, 0:1], scale=1.0 / (SC * SC),
                                 accum_out=stot[:, m * NCG + c: m * NCG + c + 1])

        def reduce_group(pairs, out_dram, tag):
            """out_dram[j] = sum over pairs (a,b) and d of (a*b)[d, j]; 256x scale."""
            row = rows.tile([1, L], dt.float32, tag=f"row_{tag}")
            for half in range(L // 512):
                rp = mmp.tile([1, 512], dt.float32, tag="mm", name=f"rp_{tag}_{half}")
                for gi, (a_view, b_view) in enumerate(pairs):
                    prod = prodp.tile([128, 512], dt.bfloat16, tag="prod")
                    nc.vector.tensor_tensor(
                        prod[:], a_view[:, half * 512:(half + 1) * 512],
                        b_view[:, half * 512:(half + 1) * 512], ALU.mult)
                    nc.tensor.matmul(rp[:], ones[:], prod[:],
                                     start=(gi == 0), stop=(gi == len(pairs) - 1))
                nc.vector.tensor_copy(row[:, half * 512:(half + 1) * 512], rp[:])
            nc.sync.dma_start(out=out_dram[:].rearrange("(one n) -> one n", one=1), in_=row[:])

        # ---- schedule: head --------------------------------------------
        project(f1d, 0, L, e1f, L, staged=fst13)
        project(f2d, 0, CW, e2f, N)            # own batch = cols 0:L
        # bf16 casts on GPSIMD (idle engine); needed only by the banded
        # terms whose row-sums run in the late sim column groups
        for d in range(2):
            nc.gpsimd.tensor_copy(e1bp[:, d * LP + PAD: d * LP + PAD + L],
                                  e1f[:, d * L: (d + 1) * L])
            nc.gpsimd.tensor_copy(e2bp[:, d * LP + PAD: d * LP + PAD + L],
                                  e2f[:, d * N: d * N + L])
        e1bd = [e1bp[:, d * LP + PAD: d * LP + PAD + L] for d in range(2)]
        e2bd = [e2bp[:, d * LP + PAD: d * LP + PAD + L] for d in range(2)]

        # ---- sim c=0/c=1 with half-chunk projection inserts -------------
        HALF = CW // 2
        sim_tile(0, 0)
        sim_tile(1, 0)
        project(f2d, 2 * HALF, HALF, e2f, N)   # chunk 1 first half
        sim_tile(2, 0)
        sim_tile(3, 0)
        project(f2d, 3 * HALF, HALF, e2f, N)
        sim_tile(4, 0)
        sim_tile(5, 0)
        project(f2d, 4 * HALF, HALF, e2f, N)   # chunk 2
        sim_tile(6, 0)
        sim_tile(7, 0)
        project(f2d, 5 * HALF, HALF, e2f, N)
        sim_tile(0, 1)
        sim_tile(1, 1)
        project(f2d, 6 * HALF, HALF, e2f, N)   # chunk 3
        sim_tile(2, 1)
        sim_tile(3, 1)
        project(f2d, 7 * HALF, HALF, e2f, N)
        for m in range(4, 8):
            sim_tile(m, 1)

        # ---- banded boxsums (DVE, after the last projection epilogue) ---
        def boxsum(pb, r, tag):
            """pb: [128, LP] padded view (zeros in pads). Returns [128, L]
            view/tile: out[:, j] = sum_{|dd|<=r} pb[:, j+PAD+dd] (clipped)."""
            wdt = 2 * r + 1
            s = {1: pb}
            for p in (2, 4, 8):
                if wdt >= p:
                    sp = band.tile([128, LP], dt.bfloat16, name=f"s{p}_{tag}",
                                   tag=f"s{p}", bufs=2)
                    h = p // 2
                    n_valid = LP - p + 1
                    nc.vector.tensor_tensor(
                        sp[:, :n_valid], s[h][:, :n_valid], s[h][:, h:h + n_valid], ALU.add)
                    s[p] = sp
            terms = _box_terms(wdt)
            t0 = PAD - r
            if len(terms) == 1:
                p0, o0 = terms[0]
                return s[p0][:, t0 + o0: t0 + o0 + L]
            acc = band.tile([128, L], dt.bfloat16, name=f"box_{tag}", tag="box", bufs=6)
            p0, o0 = terms[0]
            p1, o1 = terms[1]
            nc.vector.tensor_tensor(acc[:], s[p0][:, t0 + o0: t0 + o0 + L],
                                    s[p1][:, t0 + o1: t0 + o1 + L], ALU.add)
            for p, o in terms[2:]:
                nc.vector.tensor_tensor(acc[:], acc[:], s[p][:, t0 + o: t0 + o + L], ALU.add)
            return acc[:]

        if r_self > 0:
            bx1 = [boxsum(e1bp[:, d * LP: (d + 1) * LP], r_self, f"s1_{d}") for d in range(2)]
            bx2 = [boxsum(e2bp[:, d * LP: (d + 1) * LP], r_self, f"s2_{d}") for d in range(2)]
        if r_tgt > 0:
            if r_tgt == r_self and r_self > 0:
                bxt = bx2                      # identical boxsum, reuse
            else:
                bxt = [boxsum(e2bp[:, d * LP: (d + 1) * LP], r_tgt, f"t_{d}") for d in range(2)]

        # ---- sim c=2/c=3 with the banded row-sums slotted in ------------
        for m in range(4):
            sim_tile(m, 2)
        reduce_group(list(zip(e1bd, e2bd)), pos_main, "main")
        for m in range(4, 8):
            sim_tile(m, 2)
        if r_self > 0:
            reduce_group([(e1bd[d], bx1[d]) for d in range(2)]
                         + [(e2bd[d], bx2[d]) for d in range(2)], pos_self, "self")
        else:
            zr = rows.tile([1, L], dt.float32, tag="zr")
            nc.vector.memset(zr[:], 0.0)
            nc.sync.dma_start(out=pos_self[:].rearrange("(one n) -> one n", one=1), in_=zr[:])
        for m in range(4):
            sim_tile(m, 3)
        if r_tgt > 0:
            reduce_group([(e1bd[d], bxt[d]) for d in range(2)], pos_tgt, "tgt")
        else:
            zr2 = rows.tile([1, L], dt.float32, tag="zr2")
            nc.vector.memset(zr2[:], 0.0)
            nc.sync.dma_start(out=pos_tgt[:].rearrange("(one n) -> one n", one=1), in_=zr2[:])
        for m in range(4, 8):
            sim_tile(m, 3)

        nc.sync.dma_start(out=s_out[:], in_=stot[:])

    nc.compile()
    return nc


def kernel(feature1, feature2, W, b, positive_range_self, positive_range_tgt):
    r_self = int(np.asarray(positive_range_self))
    r_tgt = int(np.asarray(positive_range_tgt))
    assert 0 <= r_self <= PAD and 0 <= r_tgt <= PAD

    key = (r_self, r_tgt)
    if key not in _module_cache:
        _module_cache[key] = _build(r_self, r_tgt)
    nc = _module_cache[key]

    in_maps = _make_in_maps(feature1, feature2, W, b)
    res = bass_utils.run_bass_kernel_spmd(nc, in_maps, list(range(N_CORES)))

    # ---- host combine (fp64) ---------------------------------------------
    j = np.arange(L)
    s2 = SC * SC
    loss_terms = []
    for i in range(N_CORES):
        r = res.results[i]
        # S groups: stot[p, m*NCG + c]; token j = m*128 + p; sum over c groups
        S = r["s_out"].astype(np.float64).reshape(128, 8, NCG).sum(axis=2)
        S = S.T.reshape(L)                                   # token j at [j%128, j//128]
        t = KSHIFT + np.log(S) - np.log(float(N))            # negative term
        t -= r["pos_main"].astype(np.float64) / s2
        if r_self > 0:
            cnt = np.minimum(L - 1, j + r_self) - np.maximum(0, j - r_self) + 1.0
            t -= r["pos_self"].astype(np.float64) / s2 / cnt
        if r_tgt > 0:
            cnt = np.minimum(L - 1, j + r_tgt) - np.maximum(0, j - r_tgt) + 1.0
            t -= r["pos_tgt"].astype(np.float64) / s2 / cnt
        loss_terms.append(t)
    loss = np.mean(np.concatenate(loss_terms))
    return np.float32(loss)


def _swizzle(a_t):
    """[DIN, n] -> [128, KO, n] with row k at [k % 128, k // 128]."""
    n = a_t.shape[1]
    return np.ascontiguousarray(a_t.reshape(KO, 128, n).transpose(1, 0, 2))


def _make_in_maps(feature1, feature2, W, b):
    f8 = ml_dtypes.float8_e4m3fn
    f1 = np.asarray(feature1, dtype=np.float32)
    f2 = np.asarray(feature2, dtype=np.float32)
    Wr = _swizzle(np.clip(SC * np.asarray(W, dtype=np.float32), -240, 240).astype(f8))
    bv = np.ascontiguousarray(
        (SC * np.asarray(b, dtype=np.float32)).reshape(2, 128).T)
    f2t_full = np.clip(f2.reshape(N, DIN).T, -240, 240).astype(f8)   # [DIN, N]
    in_maps = []
    for i in range(N_CORES):
        f1s = _swizzle(np.clip(f1[i].T, -240, 240).astype(f8))
        f2s = _swizzle(np.roll(f2t_full, -i * L, axis=1))
        in_maps.append({"f1d": f1s, "f2d": f2s, "w_in": Wr, "b_in": bv})
    return in_maps


# revision 16
# speedup vs baseline: 1.0983x; 1.0983x over previous
"""Contrastive-learning NCE loss on 8 trn2 NeuronCores (Bass/Tile).

Problem (hardcoded shapes): B=8, L=1024, D_in=512, D_feat=256, N=B*L=8192.
  emb_k = relu(feature_k @ W + b)                     [B, L, Df]
  positive = <e1,e2> + banded_diag_mean terms         [N]
  negative = logsumexp(e1 @ e2.T, axis=-1) - log(N)   [N]
  loss = mean(-positive + negative)

Sharding: token dim N split across 8 cores = one batch row each (L == N/8).
Each core computes its [1024, 8192] slab of the similarity matrix against the
full emb_2 (recomputed locally from full feature2). The host rotates feature2
per core so the core's own batch always sits at columns 0:1023 -> the SPMD
program is core-index free.

fp8 pipeline: all matmuls are fp8e4m3 MatmulPerfMode.DoubleRow (K=256/inst,
2 elem/cycle moving stream at full p-state). Host pre-scales W,b by 16 so W
leaves the fp8 subnormal range; embeddings are stored at 16x scale. Every dot
product is 256x true scale: EXP applies scale=1/256, host divides pos_* by
256. Host inputs are pre-swizzled to [128, ko, n] so every DMA line is
contiguous per partition.

The kernel is ACT-bound (~65us/core of exp). Schedule highlights:
- warmup matmuls during the DMA head lift the PE out of the HAM throttle
  (cold 1.2 GHz -> 2.4 GHz) before the first projection lands
- constant exp shift K=48 (max sim ~120 -> args <= ~72, fp32/bf16-safe), so
  no per-row diag bias and no diag DMA round-trip on the critical path
- f2 chunks 1..3 project in 1024-col halves interleaved between sim tiles;
  with a warm PE each insert (~1.7us) fits inside ACT's 2-tile buffer
- banded terms read bf16 casts of the fp8 embeddings (lossless, so the main
  positive term matches the sim diagonal exactly); casts run on the
  otherwise-idle GPSIMD, boxsums on DVE after the last projection epilogue,
  and the row-sum matmuls slot into late sim column groups
- ACT's exp table is preloaded with a dummy activation during the DMA head
"""

import numpy as np
import ml_dtypes
from contextlib import ExitStack

import concourse.bass as bass
import concourse.tile as tile
from concourse import bacc, mybir
from concourse import bass_utils

dt = mybir.dt
AF = mybir.ActivationFunctionType
ALU = mybir.AluOpType
DR = mybir.MatmulPerfMode.DoubleRow

N_CORES = 8
B, L, DIN, DF = 8, 1024, 512, 256
N = B * L
KO = DIN // 128     # 4 k-tiles of the projection contraction
PAD = 4             # box-filter padding (max supported positive_range)
LP = L + 2 * PAD    # padded row length for banded box sums
CW = 2048           # column group width of sim/EXP tiles
NCG = N // CW       # 4 column groups
SC = 16.0           # host pre-scale on W, b; emb stored at 16x
KSHIFT = 48.0       # constant exp shift (max sim ~120 -> args <= ~72)

# Schraudolph fast-exp on DVE: exp(psum/256 - K) ~= bitcast_f32(int32(
# psum*A1 + A2)); mean rel err -0.04%, RMS 1.8% per element -> ~0.2% on a
# row sum. A subset of sim tiles is consumed this way on the otherwise-idle
# late-phase DVE, taking load off the bottleneck ACT engine.
_L2E = 1.4426950408889634
FE_A1 = 2**23 * _L2E / 256
FE_A2 = (127 - KSHIFT * _L2E) * 2**23 - 486411
DVE_TILES = ((2, 1), (1, 2), (4, 2), (1, 3), (3, 3), (5, 3))

_module_cache = {}


def _box_terms(w: int):
    """Decompose window width w (odd, <= 2*PAD+1) into power-of-2 segments:
    returns [(pow, offset), ...] s.t. window = concat of segments."""
    terms, off = [], 0
    for p in (8, 4, 2, 1):
        if w >= p:
            terms.append((p, off))
            off += p
            w -= p
    assert w == 0
    return terms


def _build(r_self: int, r_tgt: int):
    nc = bacc.Bacc("TRN2", target_bir_lowering=False, debug=False, num_devices=N_CORES)

    f1d = nc.dram_tensor("f1d", [128, KO, L], dt.float8e4, kind="ExternalInput").ap()
    f2d = nc.dram_tensor("f2d", [128, KO, N], dt.float8e4, kind="ExternalInput").ap()
    w_in = nc.dram_tensor("w_in", [128, KO, DF], dt.float8e4, kind="ExternalInput").ap()
    b_in = nc.dram_tensor("b_in", [128, 2], dt.float32, kind="ExternalInput").ap()

    pos_main = nc.dram_tensor("pos_main", [L], dt.float32, kind="ExternalOutput").ap()
    pos_self = nc.dram_tensor("pos_self", [L], dt.float32, kind="ExternalOutput").ap()
    pos_tgt = nc.dram_tensor("pos_tgt", [L], dt.float32, kind="ExternalOutput").ap()
    s_out = nc.dram_tensor("s_out", [128, 8 * NCG], dt.float32, kind="ExternalOutput").ap()
    s_outd = nc.dram_tensor("s_outd", [128, max(1, len(DVE_TILES))], dt.float32,
                            kind="ExternalOutput").ap()

    with tile.TileContext(nc) as tc, ExitStack() as ctx:
        const = ctx.enter_context(tc.tile_pool(name="const", bufs=1))
        stage = ctx.enter_context(tc.tile_pool(name="stage", bufs=3))
        emb = ctx.enter_context(tc.tile_pool(name="emb", bufs=1))
        band = ctx.enter_context(tc.tile_pool(name="band", bufs=1))
        prodp = ctx.enter_context(tc.tile_pool(name="prodp", bufs=2))
        rows = ctx.enter_context(tc.tile_pool(name="rows", bufs=1))
        esc = ctx.enter_context(tc.tile_pool(name="esc2", bufs=2))
        mmp = ctx.enter_context(tc.tile_pool(name="mmp", bufs=2, space="PSUM"))

        # ---- staged input DMAs (f1 first: it gates the first matmul) ----
        fst_f1 = stage.tile([128, KO * L], dt.float8e4, tag="fst1")
        fst13 = fst_f1[:].rearrange("p (ko n) -> p ko n", ko=KO)
        nc.sync.dma_start(out=fst13, in_=f1d[:])
        wt = const.tile([128, KO * DF], dt.float8e4)
        wt3 = wt[:].rearrange("p (ko d) -> p ko d", ko=KO)
        nc.sync.dma_start(out=wt3, in_=w_in[:])
        b_col = const.tile([128, 2], dt.float32)
        nc.sync.dma_start(out=b_col[:], in_=b_in[:])

        # ---- constants, exp-table preload, PE warmup --------------------
        ones_f = const.tile([128, 1], dt.float32)
        nc.vector.memset(ones_f[:], 1.0)
        ones = const.tile([128, 1], dt.bfloat16)
        nc.vector.tensor_copy(ones[:], ones_f[:])
        scr = const.tile([128, 512], dt.bfloat16)
        nc.vector.memset(scr[:], 0.0)
        dum = const.tile([128, 1], dt.bfloat16)
        nkb = const.tile([128, 1], dt.float32)             # -KSHIFT bias column
        nc.vector.memset(nkb[:], -KSHIFT)
        nc.scalar.activation(dum[:], ones_f[:], AF.Exp, bias=0.0, scale=1.0)
        # ~4us of dummy matmuls lift the HAM throttle before real work lands
        for w in range(2):
            warm = mmp.tile([1, 512], dt.float32, tag="mm", name=f"warm{w}")
            for _ in range(5):
                nc.tensor.matmul(warm[:], ones[:, 0:1], scr[:], start=True, stop=True)

        # ---- embedding storage -----------------------------------------
        # fp8 at 16x scale, [128, ksub, cols] layout (ksub = 2 halves of Df)
        e1f = emb.tile([128, 2 * L], dt.float8e4, name="e1f")
        e2f = emb.tile([128, 2 * N], dt.float8e4, name="e2f")
        e1f3 = e1f[:].rearrange("p (k n) -> p k n", k=2)
        e2f3 = e2f[:].rearrange("p (k n) -> p k n", k=2)
        # padded bf16 copies for the banded terms (zeros in the pads);
        # pad-strip memsets run on GPSIMD at t~6us when nothing contends
        e1bp = band.tile([128, 2 * LP], dt.bfloat16, name="e1bp")
        e2bp = band.tile([128, 2 * LP], dt.bfloat16, name="e2bp")
        for t in (e1bp, e2bp):
            for d in range(2):
                nc.gpsimd.memset(t[:, d * LP: d * LP + PAD], 0.0)
                nc.gpsimd.memset(t[:, d * LP + PAD + L: (d + 1) * LP], 0.0)

        def project(src3, col0, ncols, dstf, dstride, staged=None):
            """DoubleRow-project pre-swizzled cols [col0, col0+ncols) into the
            fp8 tile dstf at flat offset d*dstride+col0 per k-subtile d."""
            if staged is None:
                fst = stage.tile([128, KO * ncols], dt.float8e4, tag=f"fst{ncols}")
                fst3 = fst[:].rearrange("p (ko n) -> p ko n", ko=KO)
                nc.sync.dma_start(out=fst3, in_=src3[:, :, col0:col0 + ncols])
            else:
                fst3 = staged
            for d in range(2):
                ps = mmp.tile([128, ncols], dt.float32, tag="mm", name=f"pj{col0}_{d}")
                for kop in range(KO // 2):
                    for h in range(ncols // 512):
                        nc.tensor.matmul(
                            ps[:, h * 512:(h + 1) * 512],
                            wt3[:, 2 * kop:2 * kop + 2, d * 128:(d + 1) * 128],
                            fst3[:, 2 * kop:2 * kop + 2, h * 512:(h + 1) * 512],
                            start=(kop == 0), stop=(kop == KO // 2 - 1),
                            perf_mode=DR)
                nc.vector.tensor_scalar(
                    dstf[:, d * dstride + col0: d * dstride + col0 + ncols], ps[:],
                    b_col[:, d:d + 1], 0.0, ALU.add, ALU.max)

        # ---- sim tile: 4 DR matmuls + row-sum-of-exp on ACT or DVE ------
        stot = const.tile([128, 8 * NCG], dt.float32)
        stotd = const.tile([128, max(1, len(DVE_TILES))], dt.float32)

        def sim_tile(m, c):
            ps = mmp.tile([128, CW], dt.float32, tag="mm", name=f"sim{m}_{c}")
            for q in range(CW // 512):
                nc.tensor.matmul(
                    ps[:, q * 512:(q + 1) * 512],
                    e1f3[:, :, m * 128:(m + 1) * 128],
                    e2f3[:, :, c * CW + q * 512: c * CW + (q + 1) * 512],
                    start=True, stop=True, perf_mode=DR)
            if (m, c) in DVE_TILES:
                k = DVE_TILES.index((m, c))
                xi = esc.tile([128, CW], dt.int32, tag="xi")
                nc.vector.tensor_scalar(xi[:], ps[:], FE_A1, FE_A2,
                                        ALU.mult, ALU.add)
                nc.vector.tensor_reduce(stotd[:, k:k + 1],
                                        xi[:].bitcast(dt.float32),
                                        mybir.AxisListType.X, ALU.add)
            else:
                ex = esc.tile([128, CW], dt.bfloat16, tag="ex")
                nc.scalar.activation(ex[:], ps[:], AF.Exp,
                                     bias=nkb[:, 0:1], scale=1.0 / (SC * SC),
                                     accum_out=stot[:, m * NCG + c: m * NCG + c + 1])

        def reduce_group(pairs, out_dram, tag):
            """out_dram[j] = sum over pairs (a,b) and d of (a*b)[d, j]; 256x scale."""
            row = rows.tile([1, L], dt.float32, tag=f"row_{tag}")
            for half in range(L // 512):
                rp = mmp.tile([1, 512], dt.float32, tag="mm", name=f"rp_{tag}_{half}")
                for gi, (a_view, b_view) in enumerate(pairs):
                    prod = prodp.tile([128, 512], dt.bfloat16, tag="prod")
                    nc.vector.tensor_tensor(
                        prod[:], a_view[:, half * 512:(half + 1) * 512],
                        b_view[:, half * 512:(half + 1) * 512], ALU.mult)
                    nc.tensor.matmul(rp[:], ones[:], prod[:],
                                     start=(gi == 0), stop=(gi == len(pairs) - 1))
                nc.vector.tensor_copy(row[:, half * 512:(half + 1) * 512], rp[:])
            nc.sync.dma_start(out=out_dram[:].rearrange("(one n) -> one n", one=1), in_=row[:])

        # ---- schedule: head --------------------------------------------
        project(f1d, 0, L, e1f, L, staged=fst13)
        project(f2d, 0, CW, e2f, N)            # own batch = cols 0:L
        e1bd = [e1bp[:, d * LP + PAD: d * LP + PAD + L] for d in range(2)]
        e2bd = [e2bp[:, d * LP + PAD: d * LP + PAD + L] for d in range(2)]

        # ---- sim c=0/c=1 with half-chunk projection inserts -------------
        HALF = CW // 2
        sim_tile(0, 0)
        sim_tile(1, 0)
        project(f2d, 2 * HALF, HALF, e2f, N)   # chunk 1 first half
        sim_tile(2, 0)
        sim_tile(3, 0)
        project(f2d, 3 * HALF, HALF, e2f, N)
        sim_tile(4, 0)
        sim_tile(5, 0)
        project(f2d, 4 * HALF, HALF, e2f, N)   # chunk 2
        sim_tile(6, 0)
        sim_tile(7, 0)
        project(f2d, 5 * HALF, HALF, e2f, N)
        sim_tile(0, 1)
        sim_tile(1, 1)
        project(f2d, 6 * HALF, HALF, e2f, N)   # chunk 3
        sim_tile(2, 1)
        sim_tile(3, 1)
        project(f2d, 7 * HALF, HALF, e2f, N)
        for m in range(4, 8):
            sim_tile(m, 1)

        # ---- banded casts + boxsums (DVE; late priority — the scheduler
        # slots them into DVE idle gaps without displacing epilogues) -----
        for d in range(2):
            nc.vector.tensor_copy(e1bp[:, d * LP + PAD: d * LP + PAD + L],
                                  e1f[:, d * L: (d + 1) * L])
            nc.vector.tensor_copy(e2bp[:, d * LP + PAD: d * LP + PAD + L],
                                  e2f[:, d * N: d * N + L])

        def boxsum(pb, r, tag):
            """pb: [128, LP] padded view (zeros in pads). Returns [128, L]
            view/tile: out[:, j] = sum_{|dd|<=r} pb[:, j+PAD+dd] (clipped)."""
            wdt = 2 * r + 1
            s = {1: pb}
            for p in (2, 4, 8):
                if wdt >= p:
                    sp = band.tile([128, LP], dt.bfloat16, name=f"s{p}_{tag}",
                                   tag=f"s{p}", bufs=2)
                    h = p // 2
                    n_valid = LP - p + 1
                    nc.vector.tensor_tensor(
                        sp[:, :n_valid], s[h][:, :n_valid], s[h][:, h:h + n_valid], ALU.add)
                    s[p] = sp
            terms = _box_terms(wdt)
            t0 = PAD - r
            if len(terms) == 1:
                p0, o0 = terms[0]
                return s[p0][:, t0 + o0: t0 + o0 + L]
            acc = band.tile([128, L], dt.bfloat16, name=f"box_{tag}", tag="box", bufs=6)
            p0, o0 = terms[0]
            p1, o1 = terms[1]
            nc.vector.tensor_tensor(acc[:], s[p0][:, t0 + o0: t0 + o0 + L],
                                    s[p1][:, t0 + o1: t0 + o1 + L], ALU.add)
            for p, o in terms[2:]:
                nc.vector.tensor_tensor(acc[:], acc[:], s[p][:, t0 + o: t0 + o + L], ALU.add)
            return acc[:]

        if r_self > 0:
            bx1 = [boxsum(e1bp[:, d * LP: (d + 1) * LP], r_self, f"s1_{d}") for d in range(2)]
            bx2 = [boxsum(e2bp[:, d * LP: (d + 1) * LP], r_self, f"s2_{d}") for d in range(2)]
        if r_tgt > 0:
            if r_tgt == r_self and r_self > 0:
                bxt = bx2                      # identical boxsum, reuse
            else:
                bxt = [boxsum(e2bp[:, d * LP: (d + 1) * LP], r_tgt, f"t_{d}") for d in range(2)]

        # ---- sim c=2/c=3 with the banded row-sums slotted in ------------
        for m in range(4):
            sim_tile(m, 2)
        reduce_group(list(zip(e1bd, e2bd)), pos_main, "main")
        for m in range(4, 8):
            sim_tile(m, 2)
        if r_self > 0:
            reduce_group([(e1bd[d], bx1[d]) for d in range(2)]
                         + [(e2bd[d], bx2[d]) for d in range(2)], pos_self, "self")
        else:
            zr = rows.tile([1, L], dt.float32, tag="zr")
            nc.vector.memset(zr[:], 0.0)
            nc.sync.dma_start(out=pos_self[:].rearrange("(one n) -> one n", one=1), in_=zr[:])
        for m in range(4):
            sim_tile(m, 3)
        if r_tgt > 0:
            reduce_group([(e1bd[d], bxt[d]) for d in range(2)], pos_tgt, "tgt")
        else:
            zr2 = rows.tile([1, L], dt.float32, tag="zr2")
            nc.vector.memset(zr2[:], 0.0)
            nc.sync.dma_start(out=pos_tgt[:].rearrange("(one n) -> one n", one=1), in_=zr2[:])
        for m in range(4, 8):
            sim_tile(m, 3)

        nc.sync.dma_start(out=s_out[:], in_=stot[:])
        nc.sync.dma_start(out=s_outd[:], in_=stotd[:])

    nc.compile()
    return nc


def kernel(feature1, feature2, W, b, positive_range_self, positive_range_tgt):
    r_self = int(np.asarray(positive_range_self))
    r_tgt = int(np.asarray(positive_range_tgt))
    assert 0 <= r_self <= PAD and 0 <= r_tgt <= PAD

    key = (r_self, r_tgt)
    if key not in _module_cache:
        _module_cache[key] = _build(r_self, r_tgt)
    nc = _module_cache[key]

    in_maps = _make_in_maps(feature1, feature2, W, b)
    res = bass_utils.run_bass_kernel_spmd(nc, in_maps, list(range(N_CORES)))

    # ---- host combine (fp64) ---------------------------------------------
    j = np.arange(L)
    s2 = SC * SC
    loss_terms = []
    for i in range(N_CORES):
        r = res.results[i]
        # S groups: stot[p, m*NCG + c]; token j = m*128 + p. DVE-consumed
        # tiles live in s_outd (their stot slots are never written).
        Sg = r["s_out"].astype(np.float64).reshape(128, 8, NCG).copy()
        Sd = r["s_outd"].astype(np.float64)
        for k, (m, c) in enumerate(DVE_TILES):
            Sg[:, m, c] = Sd[:, k]
        S = Sg.sum(axis=2)
        S = S.T.reshape(L)                                   # token j at [j%128, j//128]
        t = KSHIFT + np.log(S) - np.log(float(N))            # negative term
        t -= r["pos_main"].astype(np.float64) / s2
        if r_self > 0:
            cnt = np.minimum(L - 1, j + r_self) - np.maximum(0, j - r_self) + 1.0
            t -= r["pos_self"].astype(np.float64) / s2 / cnt
        if r_tgt > 0:
            cnt = np.minimum(L - 1, j + r_tgt) - np.maximum(0, j - r_tgt) + 1.0
            t -= r["pos_tgt"].astype(np.float64) / s2 / cnt
        loss_terms.append(t)
    loss = np.mean(np.concatenate(loss_terms))
    return np.float32(loss)


def _swizzle(a_t):
    """[DIN, n] -> [128, KO, n] with row k at [k % 128, k // 128]."""
    n = a_t.shape[1]
    return np.ascontiguousarray(a_t.reshape(KO, 128, n).transpose(1, 0, 2))


def _make_in_maps(feature1, feature2, W, b):
    f8 = ml_dtypes.float8_e4m3fn
    f1 = np.asarray(feature1, dtype=np.float32)
    f2 = np.asarray(feature2, dtype=np.float32)
    Wr = _swizzle(np.clip(SC * np.asarray(W, dtype=np.float32), -240, 240).astype(f8))
    bv = np.ascontiguousarray(
        (SC * np.asarray(b, dtype=np.float32)).reshape(2, 128).T)
    f2t_full = np.clip(f2.reshape(N, DIN).T, -240, 240).astype(f8)   # [DIN, N]
    in_maps = []
    for i in range(N_CORES):
        f1s = _swizzle(np.clip(f1[i].T, -240, 240).astype(f8))
        f2s = _swizzle(np.roll(f2t_full, -i * L, axis=1))
        in_maps.append({"f1d": f1s, "f2d": f2s, "w_in": Wr, "b_in": bv})
    return in_maps
